# revision 1
# baseline (speedup 1.0000x reference)
"""CubePad Trainium2 kernel.

Input  x: [12, 64, 256, 256] f32  (2 cubes x 6 faces, face order F,R,B,L,T,D)
Output y: [12, 64, 258, 258] f32  (1-px border gathered from neighboring faces)

Sharding: channel-parallel across 8 cores (8 channels each); every core holds
all 12 faces so cross-face border gathers stay local. Pure SPMD, no
collectives. Everything is DRAM->DRAM HWDGE DMA:
  - per-face interior copy  (256B-row descriptors, ~99% of bytes)
  - 4 border strips + 2 corner-pair DMAs per face (4B descriptors where
    flips/column access force it)
split across the two HWDGE rings (sync=SP, scalar=ACT) and interleaved so the
tiny-descriptor strip DMAs overlap with the bandwidth-bound interior copies.
"""

import numpy as np

N_CORES = 8
NF, C_FULL, H, W = 12, 64, 256, 256
C = C_FULL // N_CORES
HP, WP = H + 2, W + 2

_ALL = slice(None)
_REV = slice(None, None, -1)
_EP = slice(None, None, H - 1)  # endpoints of a length-H strip: indices 0, H-1


def _dma_specs():
    """List of (cost, dst_idx, src_idx, extra_src_idx) index tuples applied
    identically to numpy arrays and bass APs.
    x view: [NF, C, H, W]; y view: [NF, C, HP, WP]."""
    specs = []
    for f in range(NF):
        b = 6 * (f // 6)
        i = f % 6
        F, R, B, L, T, D = b, b + 1, b + 2, b + 3, b + 4, b + 5

        t_src = [
            (T, _ALL, 255, _ALL),
            (T, _ALL, _REV, 255),
            (T, _ALL, 0, _REV),
            (T, _ALL, _ALL, 0),
            (B, _ALL, 0, _REV),
            (F, _ALL, 255, _ALL),
        ][i]
        d_src = [
            (D, _ALL, 0, _ALL),
            (D, _ALL, _ALL, 255),
            (D, _ALL, 255, _REV),
            (D, _ALL, _REV, 0),
            (F, _ALL, 0, _ALL),
            (B, _ALL, 255, _REV),
        ][i]
        l_src = [
            (L, _ALL, _ALL, 255),
            (F, _ALL, _ALL, 255),
            (R, _ALL, _ALL, 255),
            (B, _ALL, _ALL, 255),
            (L, _ALL, 0, _ALL),
            (L, _ALL, 255, _REV),
        ][i]
        r_src = [
            (R, _ALL, _ALL, 0),
            (B, _ALL, _ALL, 0),
            (L, _ALL, _ALL, 0),
            (F, _ALL, _ALL, 0),
            (R, _ALL, 0, _REV),
            (R, _ALL, 255, _ALL),
        ][i]

        def row_cost(src):
            # contiguous row source feeding a contiguous row dst -> big descs
            return 8 if src[3] == _ALL else 2048

        specs.append((row_cost(t_src), (f, _ALL, 0, slice(1, 257)), t_src, None))
        specs.append((row_cost(d_src), (f, _ALL, 257, slice(1, 257)), d_src, None))
        specs.append((2048, (f, _ALL, slice(1, 257), 0), l_src, None))
        specs.append((2048, (f, _ALL, slice(1, 257), 257), r_src, None))
        # corner pairs: endpoints of the t/d strips
        specs.append((16, (f, _ALL, 0, slice(0, 258, 257)), t_src, _EP))
        specs.append((16, (f, _ALL, 257, slice(0, 258, 257)), d_src, _EP))
    return specs


def _apply(arr, idx, extra):
    v = arr[idx]
    if extra is not None:
        v = v[:, extra]
    return v


def _build_bass():
    import concourse.bass as bass
    import concourse.mybir as mybir

    nc = bass.Bass()
    x = nc.dram_tensor("x", [NF, C, H, W], mybir.dt.float32, kind="ExternalInput")
    y = nc.dram_tensor("y", [NF, C, HP, WP], mybir.dt.float32, kind="ExternalOutput")

    specs = _dma_specs()
    # alternate strip DMAs across the two HWDGE rings
    sync_specs = [s for j, s in enumerate(specs) if j % 2 == 0]
    scalar_specs = [s for j, s in enumerate(specs) if j % 2 == 1]

    sem = nc.alloc_semaphore("dma_sem")
    n_total = len(specs) + 2  # + two interior halves

    def issue(engine, engine_specs):
        with nc.allow_non_contiguous_dma(reason="cubepad border strips"):
            for cost, dst, src, extra in engine_specs:
                dst_ap = _apply(y, dst, None)
                src_ap = _apply(x, src, extra)
                if cost == 2048:
                    # 4B-descriptor strip: make the 256-elem dim outermost so
                    # descriptors spread over all 16 SDMA engines (engine is
                    # picked by outer-dim index). Only legal when the
                    # resulting outer steps are positive.
                    dt = dst_ap.transpose([1, 0])
                    st = src_ap.transpose([1, 0])
                    if dt.ap[0][0] > 0 and st.ap[0][0] > 0:
                        dst_ap, src_ap = dt, st
                engine.dma_start(dst_ap, src_ap).then_inc(sem, 16)

    with nc.Block() as block:

        @block.sync
        def _(sync):
            # interior first half: (f,c)-outer 48 -> all 16 engines
            sync.dma_start(y[0:6, :, 1:257, 1:257],
                           x[0:6, :, :, :]).then_inc(sem, 16)
            issue(sync, sync_specs)

        @block.scalar
        def _(scalar):
            scalar.dma_start(y[6:12, :, 1:257, 1:257],
                             x[6:12, :, :, :]).then_inc(sem, 16)
            issue(scalar, scalar_specs)

    with nc.Block() as block2:

        @block2.sync
        def _(sync):
            sync.wait_ge(sem, n_total * 16)

    nc.finalize()
    return nc


_NC_CACHE = None
_TRACE = False  # set by test.py to collect an NTFF profile
_LAST_EXEC_NS = None


def kernel(x: np.ndarray) -> np.ndarray:
    global _NC_CACHE, _LAST_EXEC_NS
    from concourse.bass_utils import run_bass_kernel_spmd

    assert x.shape == (NF, C_FULL, H, W) and x.dtype == np.float32
    if _NC_CACHE is None:
        _NC_CACHE = _build_bass()
    nc = _NC_CACHE

    in_maps = [
        {"x": np.ascontiguousarray(x[:, i * C:(i + 1) * C])} for i in range(N_CORES)
    ]
    res = run_bass_kernel_spmd(
        nc, in_maps, core_ids=list(range(N_CORES)), trace=_TRACE
    )
    _LAST_EXEC_NS = res.exec_time_ns
    out = np.empty((NF, C_FULL, HP, WP), dtype=np.float32)
    for i in range(N_CORES):
        out[:, i * C:(i + 1) * C] = res.results[i]["y"]
    return out

